# revision 3
# baseline (speedup 1.0000x reference)
"""Trainium2 Bass kernel for unscaled Luong dot-product attention.

Problem: B=16, Tq=Tk=D=1024, fp32.
    scores = Q @ E^T ; weights = softmax(scores, -1) ; out = weights @ E

Sharding: data-parallel over batch — each of the 8 NeuronCores processes
2 batches end-to-end; no cross-core communication.

Layout strategy: the host-side sharding step (inside kernel()) rearranges
each core's inputs so no on-device transposition of Q or E is needed:
  - q is shipped per q-block as [qb, d-part, dc, j] (i.e. Q^T tiled), so
    each 128-row q-block's stationary operands DMA straight into SBUF.
  - e is shipped twice: natural [k-part, kc, d] (bmm2 rhs) and transposed
    [d-part, dc, k] (bmm1 rhs). One 4 MB DMA each per batch.
All three are declared float32r: the PE reads the raw fp32 bits at its
full-rate reduced internal precision (measured rel_l2 ~8e-4 vs the fp32
reference; the gate is 2e-2). Only the softmax weights, which are
produced on device, still go through a PE transpose (f32r, 1.5 cyc/row).

Per-core pipeline per batch, per 128-row q-block (software-pipelined:
block qb+1's bmm1 overlaps block qb's softmax/bmm2 tail):
  front: DMA q-block tiles, bmm1 into PSUM kh-outer (the row-max of
    half 0 starts at the halfway point).
  back: negated row-max halves (DVE) -> exp with per-partition bias and
    fused row-sum (ACT, f32r halves) -> PE-transpose W halves -> bmm2
    kc-outer (starts after the first W half) -> fold 1/rowsum into the
    PSUM->SBUF output copy (DVE) -> DMA out.
"""

import numpy as np

import concourse.bass as bass
import concourse.tile as tile
from concourse import bacc, mybir
from concourse.masks import make_identity

P = 128
B_PER_CORE = 2
T = 1024  # Tq = Tk
D = 1024
NC_CHUNKS = T // P  # 8 k-chunks / q-blocks
ND_CHUNKS = D // P  # 8 d-chunks
F32 = mybir.dt.float32
F32R = mybir.dt.float32r


def build_nc(reps: int = 1):
    nc = bacc.Bacc("TRN2", target_bir_lowering=False, debug=False)
    # q: [b, qb, p, dc*128+j] = Q[b, qb*128+j, dc*128+p]  (Q^T, block-tiled)
    q_dram = nc.dram_tensor(
        "q", [B_PER_CORE, NC_CHUNKS, P, D], F32R, kind="ExternalInput"
    ).ap()
    # e: [b, p, kc*1024+d] = E[b, kc*128+p, d]  (natural, partition-tiled)
    e_dram = nc.dram_tensor(
        "e", [B_PER_CORE, P, NC_CHUNKS * D], F32R, kind="ExternalInput"
    ).ap()
    # et: [b, p, dc*1024+k] = E[b, k, dc*128+p]  (transposed, partition-tiled)
    et_dram = nc.dram_tensor(
        "et", [B_PER_CORE, P, ND_CHUNKS * T], F32R, kind="ExternalInput"
    ).ap()
    o_dram = nc.dram_tensor("o", [B_PER_CORE, T, D], F32, kind="ExternalOutput").ap()

    with tile.TileContext(nc) as tc:
        with (
            tc.tile_pool(name="const", bufs=1) as const_pool,
            tc.tile_pool(name="e_r", bufs=2) as e_r_pool,
            tc.tile_pool(name="etr", bufs=2) as etr_pool,
            tc.tile_pool(name="qt", bufs=3) as qt_pool,
            tc.tile_pool(name="w", bufs=2) as w_pool,
            tc.tile_pool(name="wt", bufs=2) as wt_pool,
            tc.tile_pool(name="ctx", bufs=2) as ctx_pool,
            tc.tile_pool(name="stat", bufs=4) as stat_pool,
            tc.tile_pool(name="sc_ps", bufs=2, space="PSUM") as sc_psum,
            tc.tile_pool(name="ctx_ps", bufs=1, space="PSUM") as ctx_psum,
            tc.tile_pool(name="tr_ps", bufs=2, space="PSUM") as trans_psum,
        ):
            ident = const_pool.tile([P, P], F32)
            make_identity(nc, ident[:])
            ident_r = const_pool.tile([P, P], F32R)
            nc.vector.tensor_copy(ident_r[:], ident[:])

            for b in [b for _ in range(reps) for b in range(B_PER_CORE)]:
                e_r = e_r_pool.tile([P, NC_CHUNKS, D], F32R, name="e_r")
                nc.gpsimd.dma_start(e_r[:], e_dram[b])
                etr = etr_pool.tile([P, ND_CHUNKS, T], F32R, name="etr")
                nc.gpsimd.dma_start(etr[:], et_dram[b])

                def emit_front(qb, b=b, etr=etr):
                    """DMA Q^T block qb, run 1-pass f32r bmm1 kh-outer.
                    Returns the scores PSUM tile."""
                    qt = qt_pool.tile([P, ND_CHUNKS, P], F32R, name="qt")
                    nc.sync.dma_start(qt[:], q_dram[b, qb])
                    sc_ps = sc_psum.tile([P, T], F32, name="sc_ps")
                    for kh in range(2):
                        for dc in range(ND_CHUNKS):
                            nc.tensor.matmul(
                                sc_ps[:, kh * 512 : (kh + 1) * 512],
                                qt[:, dc, :],
                                etr[:, dc, kh * 512 : (kh + 1) * 512],
                                start=(dc == 0),
                                stop=(dc == ND_CHUNKS - 1),
                            )
                    return sc_ps

                def emit_back(qb, sc_ps, b=b, e_r=e_r):
                    """Softmax block qb's scores, transpose W, bmm2, store."""
                    # negated row-max per 512-half (half 0's reduce overlaps
                    # bmm1's second half), combined with min (== -max).
                    nmaxes = [
                        stat_pool.tile([P, 1], F32, tag=f"nmax{h}", name=f"nmax{h}")
                        for h in range(2)
                    ]
                    for h in range(2):
                        nc.vector.tensor_reduce(
                            out=nmaxes[h][:],
                            in_=sc_ps[:, h * 512 : (h + 1) * 512],
                            op=mybir.AluOpType.max,
                            axis=mybir.AxisListType.X,
                            negate=True,
                        )
                    negmax = stat_pool.tile([P, 1], F32, tag="negmax", name="negmax")
                    nc.vector.tensor_tensor(
                        negmax[:], nmaxes[0][:], nmaxes[1][:], mybir.AluOpType.min
                    )

                    # exp halves (f32r output) with fused row-sum accumulation,
                    # each half PE-transposed as soon as it lands.
                    w_halves = [
                        w_pool.tile([P, T // 2], F32R, tag=f"w{h}", name=f"w{h}")
                        for h in range(2)
                    ]
                    ssums = [
                        stat_pool.tile([P, 1], F32, tag=f"ssum{h}", name=f"ssum{h}")
                        for h in range(2)
                    ]
                    wt = wt_pool.tile([P, NC_CHUNKS, P], F32R, name="wt")
                    for h in range(2):
                        nc.scalar.activation(
                            w_halves[h][:],
                            sc_ps[:, h * 512 : (h + 1) * 512],
                            mybir.ActivationFunctionType.Exp,
                            bias=negmax[:],
                            accum_out=ssums[h][:],
                        )
                        tp = trans_psum.tile([P, 4 * P], F32R, name="tp")
                        for j in range(4):
                            nc.tensor.transpose(
                                tp[:, j * P : (j + 1) * P],
                                w_halves[h][:, j * P : (j + 1) * P],
                                ident_r[:],
                            )
                        nc.scalar.copy(wt[:, h * 4 : (h + 1) * 4, :], tp[:])
                    ssum = stat_pool.tile([P, 1], F32, tag="ssum", name="ssum")
                    nc.vector.tensor_tensor(
                        ssum[:], ssums[0][:], ssums[1][:], mybir.AluOpType.add
                    )
                    recip = stat_pool.tile([P, 1], F32, tag="recip", name="recip")
                    nc.vector.reciprocal(recip[:], ssum[:])

                    # bmm2: ctx[q,d] = WT.T @ E. kc outer so matmuls start
                    # once the first W half's transposes land; dh inner
                    # alternates the two PSUM banks of one [P, 1024] tile.
                    ctx_ps = ctx_psum.tile([P, T], F32, name="ctx_ps")
                    for kc in range(NC_CHUNKS):
                        for dh in range(2):
                            nc.tensor.matmul(
                                ctx_ps[:, dh * 512 : (dh + 1) * 512],
                                wt[:, kc, :],
                                e_r[:, kc, dh * 512 : (dh + 1) * 512],
                                start=(kc == 0),
                                stop=(kc == NC_CHUNKS - 1),
                            )
                    ctx_sb = ctx_pool.tile([P, D], F32, name="ctx_sb")
                    nc.vector.tensor_scalar_mul(ctx_sb[:], ctx_ps[:], recip[:])
                    nc.sync.dma_start(o_dram[b, qb * P : (qb + 1) * P, :], ctx_sb[:])

                # software pipeline: next block's bmm1 hides this block's
                # softmax + W transpose + bmm2 tail latency.
                pend = emit_front(0)
                for qb in range(NC_CHUNKS):
                    nxt = emit_front(qb + 1) if qb + 1 < NC_CHUNKS else None
                    emit_back(qb, pend)
                    pend = nxt

    nc.compile()
    return nc


def make_in_maps(decoder_hidden: np.ndarray, encoder_outputs: np.ndarray):
    """Host-side sharding + layout prep: per-core input dicts matching the
    DRAM tensor layouts declared in build_nc."""
    dh = np.asarray(decoder_hidden, dtype=np.float32)
    eo = np.asarray(encoder_outputs, dtype=np.float32)
    assert dh.shape == (16, T, D) and eo.shape == (16, T, D)
    in_maps = []
    for i in range(8):
        qc = dh[i * B_PER_CORE : (i + 1) * B_PER_CORE]
        ec = eo[i * B_PER_CORE : (i + 1) * B_PER_CORE]
        # [b, qb, j, dc, p] -> [b, qb, p, dc, j]
        qh = np.ascontiguousarray(
            qc.reshape(B_PER_CORE, NC_CHUNKS, P, ND_CHUNKS, P).transpose(0, 1, 4, 3, 2)
        ).reshape(B_PER_CORE, NC_CHUNKS, P, D)
        # [b, kc, p, d] -> [b, p, kc, d]
        eh = np.ascontiguousarray(
            ec.reshape(B_PER_CORE, NC_CHUNKS, P, D).transpose(0, 2, 1, 3)
        ).reshape(B_PER_CORE, P, NC_CHUNKS * D)
        # [b, k, dc, p] -> [b, p, dc, k]
        eth = np.ascontiguousarray(
            ec.reshape(B_PER_CORE, T, ND_CHUNKS, P).transpose(0, 3, 2, 1)
        ).reshape(B_PER_CORE, P, ND_CHUNKS * T)
        in_maps.append({"q": qh, "e": eh, "et": eth})
    return in_maps


_NC_CACHE = None


def _get_nc():
    global _NC_CACHE
    if _NC_CACHE is None:
        _NC_CACHE = build_nc()
    return _NC_CACHE


def kernel(decoder_hidden: np.ndarray, encoder_outputs: np.ndarray) -> np.ndarray:
    import os

    # The axon client here has no NTFF profiling hook; make sure a stray
    # BASS_TRACE in the environment can't push run_bass_kernel_spmd onto
    # the tracing path.
    os.environ["BASS_NEVER_TRACE"] = "1"
    from concourse import bass_utils

    nc = _get_nc()
    in_maps = make_in_maps(decoder_hidden, encoder_outputs)
    res = bass_utils.run_bass_kernel_spmd(nc, in_maps, core_ids=list(range(8)))
    return np.concatenate([r["o"] for r in res.results], axis=0)
